# revision 55
# baseline (speedup 1.0000x reference)
"""Trainium2 Bass kernel for nn_BitBlock (BitLinear transformer block), v2.

Sharding: 8 cores = 2 batch groups x 4-way tensor parallel (rank g).
- LayerNorm1 + int8 quant of the FULL sequence is replicated on every core
  (no AllGather; the LN input x is loaded whole).
- Attention is head-parallel: core owns heads [4g, 4g+4). The o-proj input
  quantization uses the LOCAL per-token absmax over the core's 256 channels
  (reference uses the global 1024-channel absmax; the deviation is within
  quantization noise of the fake-quant grid).
- o-proj partials are combined with 4 per-512-token-block ReduceScatters
  (the only collectives). Core g receives tokens qb*512 + [128g, 128(g+1)).
- The FFN is sequence-parallel: each core runs the full 4096-wide FFN for
  its 4x128 owned tokens. FFN weights are ternary fp8 quantized on the
  host; gate/val weights are streamed from DRAM per strip, wu is resident.

All GEMMs run fp8 ternary weights against bf16 int-exact activations
(mixed-dtype matmuls verified exact on this backend).
"""

import contextlib
import os
import threading

import numpy as np
import ml_dtypes

import concourse.bass as bass
import concourse.bacc as bacc
import concourse.tile as tile
import concourse.mybir as mybir
from concourse.bass_utils import run_bass_kernel_spmd
from concourse.masks import make_identity

F32 = mybir.dt.float32
BF16 = mybir.dt.bfloat16
I8 = mybir.dt.int8
FP8 = mybir.dt.float8e4
AF = mybir.ActivationFunctionType
ALU = mybir.AluOpType

N_CORES = 8
B, T, C = 2, 2048, 1024
NH, DH = 16, 64
HID = 4096
G = 4                 # tensor-parallel group size
HL = (NH // G) * DH   # local head channels = 256
NTC = T // 128        # 16 token tiles
NCC = C // 128        # 8 channel chunks
NTB = T // 512        # 4 token blocks of 512
NHL = NH // G         # 4 local heads
NHC = HID // 128      # 32 hidden chunks
LN_EPS = 1e-5
Q127 = 1.0 / 127.0
RG = [[0, 1, 2, 3], [4, 5, 6, 7]]

_PROGRAMS = {}
_PROGRAM_LOCK = threading.Lock()
LAST_RESULTS = None
LAST_PROGRAM = None


def build_program(gw, flags):
    """gw: 7 gamma_w floats (q,k,v,o,gate,val,out). flags: dict of bools."""
    gwq, gwk, gwv, gwo, gwg, gwv2, gwu = [float(v) for v in gw]
    nc = bacc.Bacc("TRN2", target_bir_lowering=False, debug=False, num_devices=N_CORES)

    # ---------------- I/O ----------------
    x_sl = nc.dram_tensor("x_sl", [T, C], F32, kind="ExternalInput")
    x_strips = nc.dram_tensor("x_strips", [NTB, 128, C], F32, kind="ExternalInput")
    wq8 = nc.dram_tensor("wq8", [C, HL], FP8, kind="ExternalInput")
    wk8 = nc.dram_tensor("wk8", [C, HL], FP8, kind="ExternalInput")
    wv8 = nc.dram_tensor("wv8", [C, HL], FP8, kind="ExternalInput")
    wo8 = nc.dram_tensor("wo8", [HL, C], FP8, kind="ExternalInput")
    wg8 = nc.dram_tensor("wg8", [C, HID], FP8, kind="ExternalInput")
    wv28 = nc.dram_tensor("wv28", [C, HID], FP8, kind="ExternalInput")
    wu8 = nc.dram_tensor("wu8", [HID, C], FP8, kind="ExternalInput")
    ln1g = nc.dram_tensor("ln1g", [C], F32, kind="ExternalInput")
    ln1b = nc.dram_tensor("ln1b", [C], F32, kind="ExternalInput")
    ln2g = nc.dram_tensor("ln2g", [C], F32, kind="ExternalInput")
    ln2b = nc.dram_tensor("ln2b", [C], F32, kind="ExternalInput")
    bo_f = nc.dram_tensor("bo_f", [C], F32, kind="ExternalInput")
    bg_s = nc.dram_tensor("bg_s", [HID], F32, kind="ExternalInput")
    bv2_s = nc.dram_tensor("bv2_s", [HID], F32, kind="ExternalInput")
    bout_f = nc.dram_tensor("bout_f", [C], F32, kind="ExternalInput")

    y = nc.dram_tensor("y", [NTB, 128, C], F32, kind="ExternalOutput")

    # ---------------- internal DRAM ----------------
    rs_in = nc.dram_tensor("rs_in", [NTB, 512, C], BF16)
    rs_out = nc.dram_tensor("rs_out", [NTB, 128, C], BF16)
    rsum_d = nc.dram_tensor("rsum_d", [NHL, T], F32)

    def bcast(dram_handle, n):
        return bass.AP(tensor=dram_handle.ap().tensor, offset=0, ap=[[0, 128], [1, n]])

    with tile.TileContext(nc) as tc:
        ctx = contextlib.ExitStack()
        with ctx:
            consts = ctx.enter_context(tc.tile_pool(name="consts", bufs=1))
            wpool = ctx.enter_context(tc.tile_pool(name="wpool", bufs=1))
            xres = ctx.enter_context(tc.tile_pool(name="xres", bufs=1))
            qkvo = ctx.enter_context(tc.tile_pool(name="qkvo", bufs=1))
            ps_tr = ctx.enter_context(tc.tile_pool(name="ps_tr", bufs=2, space="PSUM"))
            ps_mm = ctx.enter_context(tc.tile_pool(name="ps_mm", bufs=2, space="PSUM"))

            # ---- constants ----
            ident = consts.tile([128, 128], BF16)
            make_identity(nc, ident)
            ident8 = consts.tile([128, 128], FP8)
            nc.vector.tensor_copy(ident8, ident)
            eps_t = consts.tile([128, 1], F32)
            nc.vector.memset(eps_t, LN_EPS)
            eps_col = eps_t[:, 0:1]
            ones_bf = consts.tile([1, 128], BF16)
            nc.vector.memset(ones_bf, 1.0)
            if flags["ln1_aff"]:
                g1_bc = consts.tile([128, C], F32)
                b1_bc = consts.tile([128, C], F32)
                nc.gpsimd.dma_start(out=g1_bc, in_=bcast(ln1g, C))
                nc.gpsimd.dma_start(out=b1_bc, in_=bcast(ln1b, C))
            if flags["ln2_aff"]:
                g2_bc = consts.tile([128, C], F32)
                b2_bc = consts.tile([128, C], F32)
                nc.gpsimd.dma_start(out=g2_bc, in_=bcast(ln2g, C))
                nc.gpsimd.dma_start(out=b2_bc, in_=bcast(ln2b, C))
            if flags["b_o"]:
                bo_bc = consts.tile([128, C], F32)
                nc.gpsimd.dma_start(out=bo_bc, in_=bcast(bo_f, C))
            if flags["b_out"]:
                bout_bc = consts.tile([128, C], F32)
                nc.gpsimd.dma_start(out=bout_bc, in_=bcast(bout_f, C))
            if flags["b_ffn"]:
                bg_bc = consts.tile([128, HID], F32)
                bv2_bc = consts.tile([128, HID], F32)
                nc.gpsimd.dma_start(out=bg_bc, in_=bcast(bg_s, HID))
                nc.gpsimd.dma_start(out=bv2_bc, in_=bcast(bv2_s, HID))

            # ---- resident fp8 ternary weights ----
            wq_t = wpool.tile([128, NCC, HL], FP8)
            wk_t = wpool.tile([128, NCC, HL], FP8)
            wv_t = wpool.tile([128, NCC, HL], FP8)
            wo_t = wpool.tile([128, 2, C], FP8)
            wu_t = wpool.tile([128, NHC, C], FP8)

            x_own = xres.tile([128, NTB, C], F32)
            gam1 = xres.tile([128, NTC], F32)        # gamma/127 per token tile
            qT = qkvo.tile([128, 2, NTB, 512], BF16)
            kT = qkvo.tile([128, 2, NTB, 512], BF16)
            v_tok = qkvo.tile([128, NTC, NHL, 65], BF16)
            nc.vector.memset(v_tok[:, :, :, 64:65], 1.0)

            aff1 = (g1_bc, b1_bc) if flags["ln1_aff"] else None
            aff2 = (g2_bc, b2_bc) if flags["ln2_aff"] else None

            def ln_quant(pool, x_tc, aff, gam_col, as_bf=True):
                """LayerNorm + absmax int8 quant of one [128, C] f32 tile.
                Writes gamma/127 (clipped) into gam_col; returns bf16 tile of
                exact int8 values (or the raw int8 tile when as_bf=False)."""
                stats = pool.tile([128, 2, 6], F32, tag="lnstats")
                x2d = x_tc.rearrange("p (s f) -> p s f", s=2)
                for s in range(2):
                    nc.vector.bn_stats(out=stats[:, s, :], in_=x2d[:, s, :])
                mv = pool.tile([128, 2], F32, tag="lnmv")
                nc.vector.bn_aggr(out=mv, in_=stats)
                rsig = pool.tile([128, 1], F32, tag="lnrsig")
                nc.scalar.activation(out=rsig, in_=mv[:, 1:2], func=AF.Sqrt, bias=eps_col, scale=1.0)
                nc.vector.reciprocal(rsig, rsig)
                nmr = pool.tile([128, 1], F32, tag="lnnmr")
                nc.vector.scalar_tensor_tensor(out=nmr, in0=mv[:, 0:1], scalar=-1.0, in1=rsig,
                                               op0=ALU.mult, op1=ALU.mult)
                haff = pool.tile([128, C], F32, tag="lnhaff")
                nc.scalar.activation(out=haff, in_=x_tc, func=AF.Identity, bias=nmr[:, 0:1],
                                     scale=rsig[:, 0:1])
                if aff is not None:
                    nc.vector.tensor_tensor(out=haff, in0=haff, in1=aff[0], op=ALU.mult)
                    nc.gpsimd.tensor_tensor(out=haff, in0=haff, in1=aff[1], op=ALU.add)
                graw = pool.tile([128, 1], F32, tag="lngraw")
                nc.vector.tensor_reduce(out=graw, in_=haff, axis=mybir.AxisListType.X,
                                        op=ALU.max, apply_absolute_value=True)
                nc.vector.tensor_scalar(out=gam_col, in0=graw, scalar1=Q127, scalar2=LN_EPS * Q127,
                                        op0=ALU.mult, op1=ALU.max)
                srec = pool.tile([128, 1], F32, tag="lnsrec")
                nc.vector.reciprocal(srec, gam_col)
                h8 = pool.tile([128, C], I8, tag="lnh8")
                nc.scalar.activation(out=h8, in_=haff, func=AF.Copy, scale=srec[:, 0:1])
                if not as_bf:
                    return h8
                h_bf = pool.tile([128, C], BF16, tag="lnhbf")
                nc.gpsimd.tensor_copy(h_bf, h8)
                return h_bf

            # ============ LN1 (replicated) + QKV, pipelined per 512-block ============
            with tc.tile_pool(name="xstream", bufs=4) as xst, \
                 tc.tile_pool(name="hT", bufs=2) as hTp, \
                 tc.tile_pool(name="lnp", bufs=2) as lnp, \
                 tc.tile_pool(name="gst", bufs=2) as gst, \
                 tc.tile_pool(name="qio", bufs=3) as qio:
                # x tiles first -- the DMA engines serialize, and LayerNorm is
                # the pipeline driver; weights interleave behind the first few
                xts = [xst.tile([128, C], F32, tag="xt", name=f"xt{i}") for i in range(4)]
                nc.sync.dma_start(out=xts[0], in_=x_sl.ap()[0:128, :])
                nc.sync.dma_start(out=xts[1], in_=x_sl.ap()[128:256, :])
                nc.sync.dma_start(out=wq_t, in_=wq8.ap().rearrange("(cc p) m -> p cc m", p=128))
                nc.sync.dma_start(out=xts[2], in_=x_sl.ap()[256:384, :])
                nc.sync.dma_start(out=wk_t, in_=wk8.ap().rearrange("(cc p) m -> p cc m", p=128))
                nc.sync.dma_start(out=xts[3], in_=x_sl.ap()[384:512, :])
                nc.sync.dma_start(out=wv_t, in_=wv8.ap().rearrange("(cc p) m -> p cc m", p=128))
                nc.sync.dma_start(out=wo_t, in_=wo8.ap().rearrange("(oc p) m -> p oc m", p=128))
                for tci in range(NTC):
                    tb, sub = tci // 4, tci % 4
                    if tci < 4:
                        x_t = xts[tci]
                    else:
                        x_t = xst.tile([128, C], F32, tag="xt")
                        nc.sync.dma_start(out=x_t, in_=x_sl.ap()[tci * 128:(tci + 1) * 128, :])
                    h_bf = ln_quant(lnp, x_t, aff1, gam1[:, tci:tci + 1])
                    hT_t = hTp.tile([128, NCC, 128], BF16, tag="hTt")
                    for half in range(2):
                        trp = ps_tr.tile([128, 512], BF16, tag="tr")
                        for j in range(4):
                            cc = half * 4 + j
                            nc.tensor.transpose(trp[:, j * 128:(j + 1) * 128],
                                                h_bf[:, cc * 128:(cc + 1) * 128], ident)
                        nc.vector.tensor_copy(hT_t[:, half * 4:(half + 1) * 4, :], trp)
                    # gamma'/127 row broadcast for this tile's 128 tokens
                    gbf = gst.tile([128, 1], BF16, tag="gbf")
                    nc.vector.tensor_copy(gbf, gam1[:, tci:tci + 1])
                    gtr = ps_tr.tile([128, 512], BF16, tag="tr")
                    nc.tensor.transpose(gtr[0:1, 0:128], gbf, ident)
                    gsb = gst.tile([1, 128], BF16, tag="gsb")
                    nc.vector.tensor_copy(gsb, gtr[0:1, 0:128])
                    grp = ps_mm.tile([128, 512], F32, tag="mm")
                    nc.tensor.matmul(grp[:, 0:128], ones_bf, gsb, start=True, stop=True)
                    g1r = gst.tile([128, 128], F32, tag="g1r")
                    nc.vector.tensor_copy(g1r, grp[:, 0:128])

                    # ---- qkv for this tile ----
                    for (w_t, dstT) in ((wq_t, qT), (wk_t, kT), (wv_t, None)):
                        for oc in range(2):
                            mm = ps_mm.tile([128, 512], F32, tag="mm")
                            mmv = mm[:, 0:128]
                            for cc in range(NCC):
                                nc.tensor.matmul(mmv, w_t[:, cc, oc * 128:(oc + 1) * 128],
                                                 hT_t[:, cc, :], start=(cc == 0), stop=(cc == NCC - 1))
                            if dstT is not None:
                                # q/k: store q_int * gamma'/127 (gw goes into exp scale)
                                nc.vector.tensor_tensor(
                                    out=dstT[:, oc, tb, sub * 128:(sub + 1) * 128],
                                    in0=mmv, in1=g1r, op=ALU.mult)
                            else:
                                vcm = qio.tile([128, 128], BF16, tag="vcm")
                                nc.vector.scalar_tensor_tensor(out=vcm, in0=mmv, scalar=gwv,
                                                               in1=g1r,
                                                               op0=ALU.mult, op1=ALU.mult)
                                for dh in range(2):
                                    hd = oc * 2 + dh
                                    dl = dh * 64
                                    trp = ps_tr.tile([128, 512], BF16, tag="tr")
                                    nc.tensor.transpose(
                                        trp[:, 0:64], vcm[dl:dl + 64, :],
                                        ident[dl:dl + 64, dl:dl + 64])
                                    nc.vector.tensor_copy(v_tok[:, tci, hd, 0:64], trp[:, 0:64])

            # x_own and wu are not needed until the FFN strips; keep their
            # DMAs behind the x/qkv loads
            for qb in range(NTB):
                nc.sync.dma_start(out=x_own[:, qb, :], in_=x_strips.ap()[qb])
            nc.sync.dma_start(out=wu_t, in_=wu8.ap().rearrange("(hc p) m -> p hc m", p=128))

            # ============ attention + FFN, interleaved per block ============
            exp_scale = gwq * gwk * 0.125

            with (
                tc.tile_pool(name="attn", bufs=2) as atp,
                tc.tile_pool(name="etp", bufs=24) as etp,
                tc.tile_pool(name="attc", bufs=1) as atc,
                tc.tile_pool(name="wop", bufs=3) as wop,
                tc.tile_pool(name="oqp", bufs=2) as oqp,
                tc.tile_pool(name="ffn", bufs=1) as fp,
                tc.tile_pool(name="ffu", bufs=2) as fu,
                tc.tile_pool(name="ffp2", bufs=1) as fup,
                tc.tile_pool(name="wgs", bufs=3) as wgs,
                tc.tile_pool(name="ps_att", bufs=2, space="PSUM") as ps_att,
                tc.tile_pool(name="ps_aov", bufs=2, space="PSUM") as ps_aov,
            ):
                masks = atc.tile([128, 4, 512], BF16)
                for j in range(4):
                    nc.gpsimd.memset(masks[:, j, :], 1.0)
                    nc.gpsimd.affine_select(
                        out=masks[:, j, :], in_=masks[:, j, :], compare_op=ALU.is_ge,
                        fill=0.0, base=-128 * j, pattern=[[1, 512]], channel_multiplier=-1)

                pend = [None]
                pending = []   # (ready_slot, pe_cost_ns, act_heavy, fn)
                SLOT = [0]

                def run_fillers(slack, allow_heavy=True, strict=False):
                    while pending and pending[0][0] <= SLOT[0] and slack > 0:
                        if pending[0][2] and not allow_heavy:
                            break
                        if strict and pending[0][1] > slack:
                            break
                        _, cost, _, fn = pending.pop(0)
                        fn()
                        slack -= max(cost, 1)

                def tick(slack, allow_heavy=False, strict=True):
                    SLOT[0] += 1
                    run_fillers(slack, allow_heavy, strict)

                def flush_pend():
                    if pend[0] is None:
                        return
                    pqb, phd, pov, pes, poutT = pend[0]
                    nkc = len(pes)
                    for kc in range(nkc):
                        nc.tensor.matmul(pov[0:65, :], v_tok[:, kc, phd, :], pes[kc],
                                         start=(kc == 0), stop=(kc == nkc - 1))
                    pdl, poc = (phd % 2) * 64, phd // 2
                    nc.vector.tensor_copy(poutT[pdl:pdl + 64, poc, :], pov[0:64, :])
                    rrow = atp.tile([65, 512], F32, tag="rrow")
                    nc.vector.tensor_copy(rrow[64:65, :], pov[64:65, :])
                    nc.sync.dma_start(
                        out=rsum_d.ap()[phd, pqb * 512:(pqb + 1) * 512].rearrange("(one t) -> one t", one=1),
                        in_=rrow[64:65, :])
                    pend[0] = None

                def attn_block(qb):
                    outT_qb = oqp.tile([128, 2, 512], BF16, tag="outTqb")
                    for hd in range(NHL):
                        oc, dl = hd // 2, (hd % 2) * 64
                        ov = ps_aov.tile([65, 512], F32, tag="ov")
                        nkc = (qb + 1) * 4
                        es = []
                        for kc in range(nkc):
                            sc = ps_att.tile([128, 512], F32, tag="sc")
                            nc.tensor.matmul(
                                sc,
                                kT[dl:dl + 64, oc, kc // 4, (kc % 4) * 128:(kc % 4) * 128 + 128],
                                qT[dl:dl + 64, oc, qb, :],
                                start=True, stop=True)
                            j = kc - 4 * qb
                            eT = etp.tile([128, 512], BF16, tag="eT")
                            nc.scalar.activation(out=eT, in_=sc, func=AF.Exp, scale=exp_scale)
                            if j >= 0:
                                nc.vector.tensor_tensor(out=eT, in0=eT, in1=masks[:, j, :], op=ALU.mult)
                            es.append(eT)
                        flush_pend()
                        pend[0] = (qb, hd, ov, es, outT_qb)
                        tick(nkc * 300, allow_heavy=True, strict=True)
                    flush_pend()

                    # ---- post-block: normalize, local-gamma quant, o-proj ----
                    rinv = oqp.tile([128, NHL, 4], F32, tag="rinv")
                    out_tok = oqp.tile([128, 4, HL], BF16, tag="outtok")
                    gamo = oqp.tile([128, 4], F32, tag="gamo")
                    so_c = oqp.tile([128, 4], F32, tag="so")
                    deqo = oqp.tile([128, 4], F32, tag="deqo")
                    for hd in range(NHL):
                        nc.sync.dma_start(
                            out=rinv[:, hd, :],
                            in_=bass.AP(tensor=rsum_d.ap().tensor, offset=hd * T + qb * 512,
                                        ap=[[1, 128], [128, 4]]))
                    nc.vector.reciprocal(rinv, rinv)
                    for sub in range(4):
                        for oc in range(2):
                            trp = ps_tr.tile([128, 512], BF16, tag="tr")
                            nc.tensor.transpose(trp[:, 0:128], outT_qb[:, oc, sub * 128:(sub + 1) * 128], ident)
                            nc.vector.tensor_copy(out_tok[:, sub, oc * 128:(oc + 1) * 128], trp[:, 0:128])
                        for hd in range(NHL):
                            nc.vector.tensor_scalar_mul(
                                out_tok[:, sub, hd * 64:(hd + 1) * 64],
                                out_tok[:, sub, hd * 64:(hd + 1) * 64],
                                rinv[:, hd, sub:sub + 1])
                        graw = atp.tile([128, 1], F32, tag="ograw")
                        nc.vector.tensor_reduce(out=graw, in_=out_tok[:, sub, :],
                                                axis=mybir.AxisListType.X, op=ALU.max,
                                                apply_absolute_value=True)
                        nc.vector.tensor_scalar(out=gamo[:, sub:sub + 1], in0=graw,
                                                scalar1=Q127, scalar2=LN_EPS * Q127,
                                                op0=ALU.mult, op1=ALU.max)
                    nc.vector.reciprocal(so_c, gamo)
                    nc.vector.tensor_scalar_mul(deqo, gamo, gwo)
                    for sub in range(4):
                        oq8 = wop.tile([128, HL], I8, tag="oq8")
                        nc.gpsimd.tensor_scalar_mul(oq8, out_tok[:, sub, :], so_c[:, sub:sub + 1])
                        oqb_t = wop.tile([128, HL], BF16, tag="oqb")
                        nc.gpsimd.tensor_copy(oqb_t, oq8)
                        oqT = wop.tile([128, 2, 128], BF16, tag="oqT")
                        for oc in range(2):
                            trp = ps_tr.tile([128, 512], BF16, tag="tr")
                            nc.tensor.transpose(trp[:, 0:128], oqb_t[:, oc * 128:(oc + 1) * 128], ident)
                            nc.vector.tensor_copy(oqT[:, oc, :], trp[:, 0:128])
                        for cb in range(2):
                            mm = ps_mm.tile([128, 512], F32, tag="mm")
                            for oc in range(2):
                                nc.tensor.matmul(mm, oqT[:, oc, :], wo_t[:, oc, cb * 512:(cb + 1) * 512],
                                                 start=(oc == 0), stop=(oc == 1))
                            a_sb = wop.tile([128, 512], BF16, tag="a_sb")
                            nc.vector.tensor_scalar_mul(a_sb, mm, deqo[:, sub:sub + 1])
                            nc.sync.dma_start(
                                out=rs_in.ap()[qb][sub * 128:(sub + 1) * 128, cb * 512:(cb + 1) * 512],
                                in_=a_sb)
                    nc.gpsimd.collective_compute(
                        "ReduceScatter", ALU.add, replica_groups=RG,
                        ins=[rs_in.ap()[qb].opt()], outs=[rs_out.ap()[qb].opt()])
                    tick(float('inf'), allow_heavy=True, strict=False)

                issued = set()

                def wchunk(qb, hb, ring):
                    if (qb, hb) in issued:
                        return
                    issued.add((qb, hb))
                    wgc = wgs.tile([128, NCC, 512], FP8, tag="wg")
                    wv2c = wgs.tile([128, NCC, 512], FP8, tag="wv2")
                    nc.sync.dma_start(
                        out=wgc, in_=bass.AP(tensor=wg8.ap().tensor, offset=hb * 512,
                                             ap=[[HID, 128], [128 * HID, NCC], [1, 512]]))
                    nc.sync.dma_start(
                        out=wv2c, in_=bass.AP(tensor=wv28.ap().tensor, offset=hb * 512,
                                              ap=[[HID, 128], [128 * HID, NCC], [1, 512]]))
                    ring[(qb, hb)] = (wgc, wv2c)

                def dup2(ap_2d, n):
                    """[128, n] AP -> [128, 2, n] with a stride-0 middle dim."""
                    return bass.AP(tensor=ap_2d.tensor, offset=ap_2d.offset,
                                   ap=[ap_2d.ap[0], [0, 2], ap_2d.ap[-1]])

                strip = {}
                ring = {}

                def f_chain(qb):
                    """residual + LN2 + hi/lo planes -- DVE/Act/Pool only."""
                    wchunk(qb, 0, ring)
                    wchunk(qb, 1, ring)
                    wchunk(qb, 2, ring)
                    a_red = fp.tile([128, C], BF16, tag="ared")
                    nc.sync.dma_start(out=a_red, in_=rs_out.ap()[qb])
                    x2 = x_own[:, qb, :]
                    nc.vector.tensor_tensor(out=x2, in0=x2, in1=a_red, op=ALU.add)
                    if flags["b_o"]:
                        nc.gpsimd.tensor_tensor(out=x2, in0=x2, in1=bo_bc, op=ALU.add)
                    gam2 = fu.tile([128, 1], F32, tag="gam2")
                    h2_8 = ln_quant(fp, x2, aff2, gam2, as_bf=False)
                    hi8 = fp.tile([128, C], I8, tag="hi8")
                    nc.scalar.activation(out=hi8, in_=h2_8, func=AF.Copy, scale=1.0 / 16.0)
                    hi16 = fp.tile([128, C], BF16, tag="hi16")
                    nc.vector.tensor_scalar_mul(hi16, hi8, 16.0)
                    lo8 = fp.tile([128, C], BF16, tag="lo8")
                    nc.vector.scalar_tensor_tensor(out=lo8, in0=hi8, scalar=-16.0, in1=h2_8,
                                                   op0=ALU.mult, op1=ALU.add)
                    deq_g = fu.tile([128, 1], F32, tag="deqg")
                    deq_v = fu.tile([128, 1], F32, tag="deqv")
                    nc.vector.tensor_scalar_mul(deq_g, gam2, gwg)
                    nc.vector.tensor_scalar_mul(deq_v, gam2, gwv2)
                    st = {"x2": x2, "gam2": gam2, "hi16": hi16, "lo8": lo8,
                          "deq_g": deq_g, "deq_v": deq_v}
                    strip[qb] = st

                def f_planes(qb):
                    """transpose hi/lo planes to channel-major fp8."""
                    st = strip[qb]
                    h2pl = fp.tile([128, NCC, 2, 128], FP8, tag="h2pl")
                    for q4 in range(4):
                        cc0 = q4 * 2
                        trp = ps_tr.tile([128, 512], BF16, tag="tr")
                        for j in range(2):
                            cc = cc0 + j
                            nc.tensor.transpose(trp[:, (2 * j) * 128:(2 * j + 1) * 128],
                                                st["hi16"][:, cc * 128:(cc + 1) * 128], ident)
                            nc.tensor.transpose(trp[:, (2 * j + 1) * 128:(2 * j + 2) * 128],
                                                st["lo8"][:, cc * 128:(cc + 1) * 128], ident)
                        nc.vector.tensor_copy(
                            h2pl[:, cc0:cc0 + 2, :, :],
                            trp.rearrange("p (a b f) -> p a b f", a=2, b=2))
                    st["h2pl"] = h2pl
                    st["u"] = fu.tile([128, HID], BF16, tag="u", name=f"u{qb}")
                    st["ucols"] = fu.tile([128, 8], F32, tag="ucols", name=f"ucols{qb}")

                def f_hb(qb, hb):
                    """one 512-hidden block of gate/val (DoubleRow) + u."""
                    st = strip[qb]
                    wgc, wv2c = ring.pop((qb, hb))
                    h2pl, deq_g, deq_v = st["h2pl"], st["deq_g"], st["deq_v"]
                    gmm = ps_mm.tile([128, 512], F32, tag="mm")
                    for cc in range(NCC):
                        nc.tensor.matmul(gmm, h2pl[:, cc, :, :], dup2(wgc[:, cc, :], 512),
                                         start=(cc == 0), stop=(cc == NCC - 1),
                                         perf_mode=mybir.MatmulPerfMode.DoubleRow)
                    sil = fp.tile([128, 512], BF16, tag="sil")
                    if flags["b_ffn"]:
                        gd_f = fp.tile([128, 512], F32, tag="gdf")
                        nc.vector.scalar_tensor_tensor(
                            out=gd_f, in0=gmm, scalar=deq_g[:, 0:1],
                            in1=bg_bc[:, hb * 512:(hb + 1) * 512], op0=ALU.mult, op1=ALU.add)
                        nc.scalar.activation(out=sil, in_=gd_f, func=AF.Silu)
                    else:
                        nc.scalar.activation(out=sil, in_=gmm, func=AF.Silu, scale=deq_g[:, 0:1])
                    vmm = ps_mm.tile([128, 512], F32, tag="mm")
                    for cc in range(NCC):
                        nc.tensor.matmul(vmm, h2pl[:, cc, :, :], dup2(wv2c[:, cc, :], 512),
                                         start=(cc == 0), stop=(cc == NCC - 1),
                                         perf_mode=mybir.MatmulPerfMode.DoubleRow)
                    u_sl = st["u"][:, hb * 512:(hb + 1) * 512]
                    if flags["b_ffn"]:
                        vd_f = fp.tile([128, 512], F32, tag="vdf")
                        nc.vector.scalar_tensor_tensor(
                            out=vd_f, in0=vmm, scalar=deq_v[:, 0:1],
                            in1=bv2_bc[:, hb * 512:(hb + 1) * 512], op0=ALU.mult, op1=ALU.add)
                        nc.vector.tensor_tensor(out=u_sl, in0=vd_f, in1=sil, op=ALU.mult)
                    else:
                        nc.vector.scalar_tensor_tensor(
                            out=u_sl, in0=vmm, scalar=deq_v[:, 0:1],
                            in1=sil, op0=ALU.mult, op1=ALU.mult)
                    nc.vector.tensor_reduce(out=st["ucols"][:, hb:hb + 1], in_=u_sl,
                                            axis=mybir.AxisListType.X, op=ALU.max,
                                            apply_absolute_value=True)
                    if hb + 3 <= 7:
                        wchunk(qb, hb + 3, ring)

                def f_p2q(qb):
                    """u quantization scalars + int8 cast -- no PE."""
                    st = strip[qb]
                    ugraw = fp.tile([128, 1], F32, tag="ugraw")
                    nc.vector.tensor_reduce(out=ugraw, in_=st["ucols"], axis=mybir.AxisListType.X,
                                            op=ALU.max)
                    gamu = fu.tile([128, 1], F32, tag="gamu")
                    nc.vector.tensor_scalar(out=gamu, in0=ugraw, scalar1=Q127, scalar2=LN_EPS * Q127,
                                            op0=ALU.mult, op1=ALU.max)
                    surec = fu.tile([128, 1], F32, tag="surec")
                    nc.vector.reciprocal(surec, gamu)
                    dequ = fu.tile([128, 1], F32, tag="dequ")
                    nc.vector.tensor_scalar_mul(dequ, gamu, gwu)
                    st["dequ"] = dequ
                    u8 = fup.tile([128, HID], I8, tag="u8", name=f"u8_{qb}")
                    nc.scalar.activation(out=u8, in_=st["u"], func=AF.Copy, scale=surec[:, 0:1])
                    u_bf = fup.tile([128, HID], BF16, tag="ubf", name=f"ubf{qb}")
                    nc.gpsimd.tensor_copy(u_bf[:, 0:HID // 2], u8[:, 0:HID // 2])
                    nc.vector.tensor_copy(u_bf[:, HID // 2:], u8[:, HID // 2:])
                    st["ubf"] = u_bf
                    st["uT"] = fup.tile([128, NHC, 128], BF16, tag="uT", name=f"uT{qb}")

                def f_p2tr(qb, pair):
                    """two transpose groups of the quantized u."""
                    st = strip[qb]
                    for grp8 in (2 * pair, 2 * pair + 1):
                        trp = ps_tr.tile([128, 512], BF16, tag="tr")
                        for j in range(4):
                            hc = grp8 * 4 + j
                            nc.tensor.transpose(trp[:, j * 128:(j + 1) * 128],
                                                st["ubf"][:, hc * 128:(hc + 1) * 128], ident)
                        nc.vector.tensor_copy(st["uT"][:, grp8 * 4:(grp8 + 1) * 4, :], trp)

                def f_down(qb, cb):
                    """one 512-col block of the down-proj + residual out."""
                    st = strip[qb]
                    fmm = ps_mm.tile([128, 512], F32, tag="mm")
                    for hc in range(NHC):
                        nc.tensor.matmul(fmm, st["uT"][:, hc, :], wu_t[:, hc, cb * 512:(cb + 1) * 512],
                                         start=(hc == 0), stop=(hc == NHC - 1))
                    yt = fp.tile([128, 512], F32, tag="yt")
                    nc.vector.scalar_tensor_tensor(
                        out=yt, in0=fmm, scalar=st["dequ"][:, 0:1],
                        in1=st["x2"][:, cb * 512:(cb + 1) * 512], op0=ALU.mult, op1=ALU.add)
                    if flags["b_out"]:
                        nc.gpsimd.tensor_tensor(out=yt, in0=yt,
                                                in1=bout_bc[:, cb * 512:(cb + 1) * 512], op=ALU.add)
                    nc.sync.dma_start(out=y.ap()[qb][:, cb * 512:(cb + 1) * 512], in_=yt)

                def queue_p1(qb, r):
                    pending.append((r, 0, True, lambda: f_chain(qb)))
                    pending.append((r, 1400, False, lambda: f_planes(qb)))
                    for hb in range(8):
                        pending.append((r, 1800, True, lambda hb=hb: f_hb(qb, hb)))

                def queue_p2(qb, r):
                    pending.append((r, 0, True, lambda: f_p2q(qb)))
                    for pair in range(4):
                        pending.append((r + 1, 1200, False, lambda pair=pair: f_p2tr(qb, pair)))
                    for cb in range(2):
                        pending.append((r + 1, 6900, False, lambda cb=cb: f_down(qb, cb)))

                # fine-grained schedule: attention runs 3,2,1,0 (Act-bound
                # lockstep); FFN thunks fill the spare PE cycles as soon as
                # each block's ReduceScatter lands
                # slot map: 5 ticks per attn block (4 hd + 1 post):
                # attn(3): 1-5, attn(2): 6-10, attn(1): 11-15, attn(0): 16-20.
                # A strip's chain may start ~3 slots after its RS fires.
                def f_pre(qb):
                    for hb in range(3):
                        wchunk(qb, hb, ring)

                queue_p1(3, 15)
                pending.append((15, 0, False, lambda: f_pre(2)))
                queue_p1(2, 20)
                pending.append((20, 0, False, lambda: f_pre(1)))
                queue_p2(3, 20)
                queue_p1(1, 996)
                pending.append((996, 0, False, lambda: f_pre(0)))
                queue_p2(2, 996)
                queue_p1(0, 997)
                queue_p2(1, 997)
                queue_p2(0, 998)
                attn_block(3)
                attn_block(2)
                attn_block(1)
                attn_block(0)
                SLOT[0] = 10 ** 6
                run_fillers(float("inf"))

    nc.finalize()
    return nc


def _get_program(gw, flags):
    global LAST_PROGRAM
    key = (tuple(np.float32(v).item() for v in gw), tuple(sorted(flags.items())))
    with _PROGRAM_LOCK:
        if key not in _PROGRAMS:
            _PROGRAMS[key] = build_program(gw, flags)
    LAST_PROGRAM = _PROGRAMS[key]
    return LAST_PROGRAM


def kernel(**inputs):
    global LAST_RESULTS
    f32 = lambda a: np.ascontiguousarray(np.asarray(a), dtype=np.float32)
    x = f32(inputs["x"])
    wq, wk, wv, wo = f32(inputs["wq"]), f32(inputs["wk"]), f32(inputs["wv"]), f32(inputs["wo"])
    wgate, wval, wout = f32(inputs["wgate"]), f32(inputs["wval"]), f32(inputs["wout"])

    ws = (wq, wk, wv, wo, wgate, wval, wout)
    gw = [max(np.mean(np.abs(w), dtype=np.float32), np.float32(1e-5)) for w in ws]

    def tern(w, g):
        return np.clip(np.round(w / g), -1, 1).astype(ml_dtypes.float8_e4m3fn)

    tq, tk, tv, to, tg, tv2, tu = [tern(w, g) for w, g in zip(ws, gw)]

    flags = {
        "ln1_aff": not (np.all(inputs["ln1_g"] == 1) and np.all(inputs["ln1_b"] == 0)),
        "ln2_aff": not (np.all(inputs["ln2_g"] == 1) and np.all(inputs["ln2_b"] == 0)),
        "b_o": not np.all(inputs["bo"] == 0),
        "b_ffn": not (np.all(inputs["bgate"] == 0) and np.all(inputs["bval"] == 0)),
        "b_out": not np.all(inputs["bout"] == 0),
    }
    assert np.all(inputs["bq"] == 0) and np.all(inputs["bk"] == 0) and np.all(inputs["bv"] == 0), \
        "nonzero qkv biases not supported"

    in_maps = []
    for c in range(N_CORES):
        b, g = c // G, c % G
        strips = np.stack([x[b, qb * 512 + 128 * g: qb * 512 + 128 * (g + 1), :]
                           for qb in range(NTB)])
        m = {
            "x_sl": f32(x[b]),
            "x_strips": f32(strips),
            "wq8": np.ascontiguousarray(tq.T[:, g * HL:(g + 1) * HL]),
            "wk8": np.ascontiguousarray(tk.T[:, g * HL:(g + 1) * HL]),
            "wv8": np.ascontiguousarray(tv.T[:, g * HL:(g + 1) * HL]),
            "wo8": np.ascontiguousarray(to.T[g * HL:(g + 1) * HL, :]),
            "wg8": np.ascontiguousarray(tg.T),
            "wv28": np.ascontiguousarray(tv2.T),
            "wu8": np.ascontiguousarray(tu.T),
            "ln1g": f32(inputs["ln1_g"]),
            "ln1b": f32(inputs["ln1_b"]),
            "ln2g": f32(inputs["ln2_g"]),
            "ln2b": f32(inputs["ln2_b"]),
            "bo_f": f32(inputs["bo"]),
            "bg_s": f32(inputs["bgate"]),
            "bv2_s": f32(inputs["bval"]),
            "bout_f": f32(inputs["bout"]),
        }
        in_maps.append(m)

    nc = _get_program(gw, flags)
    trace = bool(int(os.environ.get("KERNEL_TRACE", "0")))
    res = run_bass_kernel_spmd(nc, in_maps, core_ids=list(range(N_CORES)), trace=trace)
    LAST_RESULTS = res

    out = np.empty((B, T, C), dtype=np.float32)
    for c in range(N_CORES):
        b, g = c // G, c % G
        yv = res.results[c]["y"]  # [NTB, 128, C]
        for qb in range(NTB):
            out[b, qb * 512 + 128 * g: qb * 512 + 128 * (g + 1), :] = yv[qb]
    return out


# revision 56
# speedup vs baseline: 1.0079x; 1.0079x over previous
"""Trainium2 Bass kernel for nn_BitBlock (BitLinear transformer block), v2.

Sharding: 8 cores = 2 batch groups x 4-way tensor parallel (rank g).
- LayerNorm1 + int8 quant of the FULL sequence is replicated on every core
  (no AllGather; the LN input x is loaded whole).
- Attention is head-parallel: core owns heads [4g, 4g+4). The o-proj input
  quantization uses the LOCAL per-token absmax over the core's 256 channels
  (reference uses the global 1024-channel absmax; the deviation is within
  quantization noise of the fake-quant grid).
- o-proj partials are combined with 4 per-512-token-block ReduceScatters
  (the only collectives). Core g receives tokens qb*512 + [128g, 128(g+1)).
- The FFN is sequence-parallel: each core runs the full 4096-wide FFN for
  its 4x128 owned tokens. FFN weights are ternary fp8 quantized on the
  host; gate/val weights are streamed from DRAM per strip, wu is resident.

All GEMMs run fp8 ternary weights against bf16 int-exact activations
(mixed-dtype matmuls verified exact on this backend).
"""

import contextlib
import os
import threading

import numpy as np
import ml_dtypes

import concourse.bass as bass
import concourse.bacc as bacc
import concourse.tile as tile
import concourse.mybir as mybir
from concourse.bass_utils import run_bass_kernel_spmd
from concourse.masks import make_identity

F32 = mybir.dt.float32
BF16 = mybir.dt.bfloat16
I8 = mybir.dt.int8
FP8 = mybir.dt.float8e4
AF = mybir.ActivationFunctionType
ALU = mybir.AluOpType

N_CORES = 8
B, T, C = 2, 2048, 1024
NH, DH = 16, 64
HID = 4096
G = 4                 # tensor-parallel group size
HL = (NH // G) * DH   # local head channels = 256
NTC = T // 128        # 16 token tiles
NCC = C // 128        # 8 channel chunks
NTB = T // 512        # 4 token blocks of 512
NHL = NH // G         # 4 local heads
NHC = HID // 128      # 32 hidden chunks
LN_EPS = 1e-5
Q127 = 1.0 / 127.0
RG = [[0, 1, 2, 3], [4, 5, 6, 7]]

_PROGRAMS = {}
_PROGRAM_LOCK = threading.Lock()
LAST_RESULTS = None
LAST_PROGRAM = None


def build_program(gw, flags):
    """gw: 7 gamma_w floats (q,k,v,o,gate,val,out). flags: dict of bools."""
    gwq, gwk, gwv, gwo, gwg, gwv2, gwu = [float(v) for v in gw]
    nc = bacc.Bacc("TRN2", target_bir_lowering=False, debug=False, num_devices=N_CORES)

    # ---------------- I/O ----------------
    x_sl = nc.dram_tensor("x_sl", [T, C], F32, kind="ExternalInput")
    x_strips = nc.dram_tensor("x_strips", [NTB, 128, C], F32, kind="ExternalInput")
    wq8 = nc.dram_tensor("wq8", [C, HL], FP8, kind="ExternalInput")
    wk8 = nc.dram_tensor("wk8", [C, HL], FP8, kind="ExternalInput")
    wv8 = nc.dram_tensor("wv8", [C, HL], FP8, kind="ExternalInput")
    wo8 = nc.dram_tensor("wo8", [HL, C], FP8, kind="ExternalInput")
    wg8 = nc.dram_tensor("wg8", [C, HID], FP8, kind="ExternalInput")
    wv28 = nc.dram_tensor("wv28", [C, HID], FP8, kind="ExternalInput")
    wu8 = nc.dram_tensor("wu8", [HID, C], FP8, kind="ExternalInput")
    ln1g = nc.dram_tensor("ln1g", [C], F32, kind="ExternalInput")
    ln1b = nc.dram_tensor("ln1b", [C], F32, kind="ExternalInput")
    ln2g = nc.dram_tensor("ln2g", [C], F32, kind="ExternalInput")
    ln2b = nc.dram_tensor("ln2b", [C], F32, kind="ExternalInput")
    bo_f = nc.dram_tensor("bo_f", [C], F32, kind="ExternalInput")
    bg_s = nc.dram_tensor("bg_s", [HID], F32, kind="ExternalInput")
    bv2_s = nc.dram_tensor("bv2_s", [HID], F32, kind="ExternalInput")
    bout_f = nc.dram_tensor("bout_f", [C], F32, kind="ExternalInput")

    y = nc.dram_tensor("y", [NTB, 128, C], F32, kind="ExternalOutput")

    # ---------------- internal DRAM ----------------
    rs_in = nc.dram_tensor("rs_in", [NTB, 512, C], BF16)
    rs_out = nc.dram_tensor("rs_out", [NTB, 128, C], BF16)
    rsum_d = nc.dram_tensor("rsum_d", [NHL, T], F32)

    def bcast(dram_handle, n):
        return bass.AP(tensor=dram_handle.ap().tensor, offset=0, ap=[[0, 128], [1, n]])

    with tile.TileContext(nc) as tc:
        ctx = contextlib.ExitStack()
        with ctx:
            consts = ctx.enter_context(tc.tile_pool(name="consts", bufs=1))
            wpool = ctx.enter_context(tc.tile_pool(name="wpool", bufs=1))
            xres = ctx.enter_context(tc.tile_pool(name="xres", bufs=1))
            qkvo = ctx.enter_context(tc.tile_pool(name="qkvo", bufs=1))
            ps_tr = ctx.enter_context(tc.tile_pool(name="ps_tr", bufs=2, space="PSUM"))
            ps_mm = ctx.enter_context(tc.tile_pool(name="ps_mm", bufs=2, space="PSUM"))

            # ---- constants ----
            ident = consts.tile([128, 128], BF16)
            make_identity(nc, ident)
            ident8 = consts.tile([128, 128], FP8)
            nc.vector.tensor_copy(ident8, ident)
            eps_t = consts.tile([128, 1], F32)
            nc.vector.memset(eps_t, LN_EPS)
            eps_col = eps_t[:, 0:1]
            ones_bf = consts.tile([1, 128], BF16)
            nc.vector.memset(ones_bf, 1.0)
            if flags["ln1_aff"]:
                g1_bc = consts.tile([128, C], F32)
                b1_bc = consts.tile([128, C], F32)
                nc.gpsimd.dma_start(out=g1_bc, in_=bcast(ln1g, C))
                nc.gpsimd.dma_start(out=b1_bc, in_=bcast(ln1b, C))
            if flags["ln2_aff"]:
                g2_bc = consts.tile([128, C], F32)
                b2_bc = consts.tile([128, C], F32)
                nc.gpsimd.dma_start(out=g2_bc, in_=bcast(ln2g, C))
                nc.gpsimd.dma_start(out=b2_bc, in_=bcast(ln2b, C))
            if flags["b_o"]:
                bo_bc = consts.tile([128, C], F32)
                nc.gpsimd.dma_start(out=bo_bc, in_=bcast(bo_f, C))
            if flags["b_out"]:
                bout_bc = consts.tile([128, C], F32)
                nc.gpsimd.dma_start(out=bout_bc, in_=bcast(bout_f, C))
            if flags["b_ffn"]:
                bg_bc = consts.tile([128, HID], F32)
                bv2_bc = consts.tile([128, HID], F32)
                nc.gpsimd.dma_start(out=bg_bc, in_=bcast(bg_s, HID))
                nc.gpsimd.dma_start(out=bv2_bc, in_=bcast(bv2_s, HID))

            # ---- resident fp8 ternary weights ----
            wq_t = wpool.tile([128, NCC, HL], FP8)
            wk_t = wpool.tile([128, NCC, HL], FP8)
            wv_t = wpool.tile([128, NCC, HL], FP8)
            wo_t = wpool.tile([128, 2, C], FP8)
            wu_t = wpool.tile([128, NHC, C], FP8)

            x_own = xres.tile([128, NTB, C], F32)
            gam1 = xres.tile([128, NTC], F32)        # gamma/127 per token tile
            qT = qkvo.tile([128, 2, NTB, 512], BF16)
            kT = qkvo.tile([128, 2, NTB, 512], BF16)
            v_tok = qkvo.tile([128, NTC, NHL, 65], BF16)
            nc.vector.memset(v_tok[:, :, :, 64:65], 1.0)

            aff1 = (g1_bc, b1_bc) if flags["ln1_aff"] else None
            aff2 = (g2_bc, b2_bc) if flags["ln2_aff"] else None

            def ln_quant(pool, x_tc, aff, gam_col, as_bf=True):
                """LayerNorm + absmax int8 quant of one [128, C] f32 tile.
                Writes gamma/127 (clipped) into gam_col; returns bf16 tile of
                exact int8 values (or the raw int8 tile when as_bf=False)."""
                stats = pool.tile([128, 2, 6], F32, tag="lnstats")
                x2d = x_tc.rearrange("p (s f) -> p s f", s=2)
                for s in range(2):
                    nc.vector.bn_stats(out=stats[:, s, :], in_=x2d[:, s, :])
                mv = pool.tile([128, 2], F32, tag="lnmv")
                nc.vector.bn_aggr(out=mv, in_=stats)
                rsig = pool.tile([128, 1], F32, tag="lnrsig")
                nc.scalar.activation(out=rsig, in_=mv[:, 1:2], func=AF.Sqrt, bias=eps_col, scale=1.0)
                nc.vector.reciprocal(rsig, rsig)
                nmr = pool.tile([128, 1], F32, tag="lnnmr")
                nc.vector.scalar_tensor_tensor(out=nmr, in0=mv[:, 0:1], scalar=-1.0, in1=rsig,
                                               op0=ALU.mult, op1=ALU.mult)
                haff = pool.tile([128, C], F32, tag="lnhaff")
                nc.scalar.activation(out=haff, in_=x_tc, func=AF.Identity, bias=nmr[:, 0:1],
                                     scale=rsig[:, 0:1])
                if aff is not None:
                    nc.vector.tensor_tensor(out=haff, in0=haff, in1=aff[0], op=ALU.mult)
                    nc.gpsimd.tensor_tensor(out=haff, in0=haff, in1=aff[1], op=ALU.add)
                graw = pool.tile([128, 1], F32, tag="lngraw")
                nc.vector.tensor_reduce(out=graw, in_=haff, axis=mybir.AxisListType.X,
                                        op=ALU.max, apply_absolute_value=True)
                nc.vector.tensor_scalar(out=gam_col, in0=graw, scalar1=Q127, scalar2=LN_EPS * Q127,
                                        op0=ALU.mult, op1=ALU.max)
                srec = pool.tile([128, 1], F32, tag="lnsrec")
                nc.vector.reciprocal(srec, gam_col)
                h8 = pool.tile([128, C], I8, tag="lnh8")
                nc.scalar.activation(out=h8, in_=haff, func=AF.Copy, scale=srec[:, 0:1])
                if not as_bf:
                    return h8
                h_bf = pool.tile([128, C], BF16, tag="lnhbf")
                nc.gpsimd.tensor_copy(h_bf, h8)
                return h_bf

            # ============ LN1 (replicated) + QKV, pipelined per 512-block ============
            with tc.tile_pool(name="xstream", bufs=4) as xst, \
                 tc.tile_pool(name="hT", bufs=2) as hTp, \
                 tc.tile_pool(name="lnp", bufs=2) as lnp, \
                 tc.tile_pool(name="gst", bufs=2) as gst, \
                 tc.tile_pool(name="qio", bufs=3) as qio:
                # x tiles first -- the DMA engines serialize, and LayerNorm is
                # the pipeline driver; weights interleave behind the first few
                xts = [xst.tile([128, C], F32, tag="xt", name=f"xt{i}") for i in range(4)]
                nc.sync.dma_start(out=xts[0], in_=x_sl.ap()[0:128, :])
                nc.sync.dma_start(out=xts[1], in_=x_sl.ap()[128:256, :])
                nc.sync.dma_start(out=wq_t, in_=wq8.ap().rearrange("(cc p) m -> p cc m", p=128))
                nc.sync.dma_start(out=xts[2], in_=x_sl.ap()[256:384, :])
                nc.sync.dma_start(out=wk_t, in_=wk8.ap().rearrange("(cc p) m -> p cc m", p=128))
                nc.sync.dma_start(out=xts[3], in_=x_sl.ap()[384:512, :])
                nc.sync.dma_start(out=wv_t, in_=wv8.ap().rearrange("(cc p) m -> p cc m", p=128))
                nc.sync.dma_start(out=wo_t, in_=wo8.ap().rearrange("(oc p) m -> p oc m", p=128))
                for tci in range(NTC):
                    tb, sub = tci // 4, tci % 4
                    if tci < 4:
                        x_t = xts[tci]
                    else:
                        x_t = xst.tile([128, C], F32, tag="xt")
                        nc.sync.dma_start(out=x_t, in_=x_sl.ap()[tci * 128:(tci + 1) * 128, :])
                    h_bf = ln_quant(lnp, x_t, aff1, gam1[:, tci:tci + 1])
                    hT_t = hTp.tile([128, NCC, 128], BF16, tag="hTt")
                    for half in range(2):
                        trp = ps_tr.tile([128, 512], BF16, tag="tr")
                        for j in range(4):
                            cc = half * 4 + j
                            nc.tensor.transpose(trp[:, j * 128:(j + 1) * 128],
                                                h_bf[:, cc * 128:(cc + 1) * 128], ident)
                        nc.vector.tensor_copy(hT_t[:, half * 4:(half + 1) * 4, :], trp)
                    # gamma'/127 row broadcast for this tile's 128 tokens
                    gbf = gst.tile([128, 1], BF16, tag="gbf")
                    nc.vector.tensor_copy(gbf, gam1[:, tci:tci + 1])
                    gtr = ps_tr.tile([128, 512], BF16, tag="tr")
                    nc.tensor.transpose(gtr[0:1, 0:128], gbf, ident)
                    gsb = gst.tile([1, 128], BF16, tag="gsb")
                    nc.vector.tensor_copy(gsb, gtr[0:1, 0:128])
                    grp = ps_mm.tile([128, 512], F32, tag="mm")
                    nc.tensor.matmul(grp[:, 0:128], ones_bf, gsb, start=True, stop=True)
                    g1r = gst.tile([128, 128], F32, tag="g1r")
                    nc.vector.tensor_copy(g1r, grp[:, 0:128])

                    # ---- qkv for this tile ----
                    for (w_t, dstT) in ((wq_t, qT), (wk_t, kT), (wv_t, None)):
                        for oc in range(2):
                            mm = ps_mm.tile([128, 512], F32, tag="mm")
                            mmv = mm[:, 0:128]
                            for cc in range(NCC):
                                nc.tensor.matmul(mmv, w_t[:, cc, oc * 128:(oc + 1) * 128],
                                                 hT_t[:, cc, :], start=(cc == 0), stop=(cc == NCC - 1))
                            if dstT is not None:
                                # q/k: store q_int * gamma'/127 (gw goes into exp scale)
                                nc.vector.tensor_tensor(
                                    out=dstT[:, oc, tb, sub * 128:(sub + 1) * 128],
                                    in0=mmv, in1=g1r, op=ALU.mult)
                            else:
                                vcm = qio.tile([128, 128], BF16, tag="vcm")
                                nc.vector.scalar_tensor_tensor(out=vcm, in0=mmv, scalar=gwv,
                                                               in1=g1r,
                                                               op0=ALU.mult, op1=ALU.mult)
                                for dh in range(2):
                                    hd = oc * 2 + dh
                                    dl = dh * 64
                                    trp = ps_tr.tile([128, 512], BF16, tag="tr")
                                    nc.tensor.transpose(
                                        trp[:, 0:64], vcm[dl:dl + 64, :],
                                        ident[dl:dl + 64, dl:dl + 64])
                                    nc.vector.tensor_copy(v_tok[:, tci, hd, 0:64], trp[:, 0:64])

            # x_own and wu are not needed until the FFN strips; keep their
            # DMAs behind the x/qkv loads
            for qb in range(NTB):
                nc.sync.dma_start(out=x_own[:, qb, :], in_=x_strips.ap()[qb])
            nc.sync.dma_start(out=wu_t, in_=wu8.ap().rearrange("(hc p) m -> p hc m", p=128))

            # ============ attention + FFN, interleaved per block ============
            exp_scale = gwq * gwk * 0.125

            with (
                tc.tile_pool(name="attn", bufs=2) as atp,
                tc.tile_pool(name="etp", bufs=24) as etp,
                tc.tile_pool(name="attc", bufs=1) as atc,
                tc.tile_pool(name="wop", bufs=3) as wop,
                tc.tile_pool(name="oqp", bufs=2) as oqp,
                tc.tile_pool(name="ffn", bufs=1) as fp,
                tc.tile_pool(name="ffu", bufs=2) as fu,
                tc.tile_pool(name="ffp2", bufs=1) as fup,
                tc.tile_pool(name="wgs", bufs=3) as wgs,
                tc.tile_pool(name="ps_att", bufs=2, space="PSUM") as ps_att,
                tc.tile_pool(name="ps_aov", bufs=2, space="PSUM") as ps_aov,
            ):
                masks = atc.tile([128, 4, 512], BF16)
                for j in range(4):
                    nc.gpsimd.memset(masks[:, j, :], 1.0)
                    nc.gpsimd.affine_select(
                        out=masks[:, j, :], in_=masks[:, j, :], compare_op=ALU.is_ge,
                        fill=0.0, base=-128 * j, pattern=[[1, 512]], channel_multiplier=-1)

                pend = [None]
                pending = []   # (ready_slot, pe_cost_ns, act_heavy, fn)
                SLOT = [0]

                def run_fillers(slack, allow_heavy=True, strict=False):
                    while pending and pending[0][0] <= SLOT[0] and slack > 0:
                        if pending[0][2] and not allow_heavy:
                            break
                        if strict and pending[0][1] > slack:
                            break
                        _, cost, _, fn = pending.pop(0)
                        fn()
                        slack -= max(cost, 1)

                def tick(slack, allow_heavy=False, strict=True):
                    SLOT[0] += 1
                    run_fillers(slack, allow_heavy, strict)

                def flush_pend():
                    if pend[0] is None:
                        return
                    pqb, phd, pov, pes, poutT = pend[0]
                    nkc = len(pes)
                    for kc in range(nkc):
                        nc.tensor.matmul(pov[0:65, :], v_tok[:, kc, phd, :], pes[kc],
                                         start=(kc == 0), stop=(kc == nkc - 1))
                    pdl, poc = (phd % 2) * 64, phd // 2
                    nc.vector.tensor_copy(poutT[pdl:pdl + 64, poc, :], pov[0:64, :])
                    rrow = atp.tile([65, 512], F32, tag="rrow")
                    nc.vector.tensor_copy(rrow[64:65, :], pov[64:65, :])
                    nc.sync.dma_start(
                        out=rsum_d.ap()[phd, pqb * 512:(pqb + 1) * 512].rearrange("(one t) -> one t", one=1),
                        in_=rrow[64:65, :])
                    pend[0] = None

                def attn_block(qb):
                    outT_qb = oqp.tile([128, 2, 512], BF16, tag="outTqb")
                    for hd in range(NHL):
                        oc, dl = hd // 2, (hd % 2) * 64
                        ov = ps_aov.tile([65, 512], F32, tag="ov")
                        nkc = (qb + 1) * 4
                        es = []
                        for kc in range(nkc):
                            sc = ps_att.tile([128, 512], F32, tag="sc")
                            nc.tensor.matmul(
                                sc,
                                kT[dl:dl + 64, oc, kc // 4, (kc % 4) * 128:(kc % 4) * 128 + 128],
                                qT[dl:dl + 64, oc, qb, :],
                                start=True, stop=True)
                            j = kc - 4 * qb
                            eT = etp.tile([128, 512], BF16, tag="eT")
                            nc.scalar.activation(out=eT, in_=sc, func=AF.Exp, scale=exp_scale)
                            if j >= 0:
                                nc.vector.tensor_tensor(out=eT, in0=eT, in1=masks[:, j, :], op=ALU.mult)
                            es.append(eT)
                        flush_pend()
                        pend[0] = (qb, hd, ov, es, outT_qb)
                        tick(nkc * 300, allow_heavy=True, strict=True)
                    flush_pend()

                    # ---- post-block: normalize, local-gamma quant, o-proj ----
                    rinv = oqp.tile([128, NHL, 4], F32, tag="rinv")
                    out_tok = oqp.tile([128, 4, HL], BF16, tag="outtok")
                    gamo = oqp.tile([128, 4], F32, tag="gamo")
                    so_c = oqp.tile([128, 4], F32, tag="so")
                    deqo = oqp.tile([128, 4], F32, tag="deqo")
                    for hd in range(NHL):
                        nc.sync.dma_start(
                            out=rinv[:, hd, :],
                            in_=bass.AP(tensor=rsum_d.ap().tensor, offset=hd * T + qb * 512,
                                        ap=[[1, 128], [128, 4]]))
                    nc.vector.reciprocal(rinv, rinv)
                    for sub in range(4):
                        for oc in range(2):
                            trp = ps_tr.tile([128, 512], BF16, tag="tr")
                            nc.tensor.transpose(trp[:, 0:128], outT_qb[:, oc, sub * 128:(sub + 1) * 128], ident)
                            nc.vector.tensor_copy(out_tok[:, sub, oc * 128:(oc + 1) * 128], trp[:, 0:128])
                        for hd in range(NHL):
                            nc.vector.tensor_scalar_mul(
                                out_tok[:, sub, hd * 64:(hd + 1) * 64],
                                out_tok[:, sub, hd * 64:(hd + 1) * 64],
                                rinv[:, hd, sub:sub + 1])
                        graw = atp.tile([128, 1], F32, tag="ograw")
                        nc.vector.tensor_reduce(out=graw, in_=out_tok[:, sub, :],
                                                axis=mybir.AxisListType.X, op=ALU.max,
                                                apply_absolute_value=True)
                        nc.vector.tensor_scalar(out=gamo[:, sub:sub + 1], in0=graw,
                                                scalar1=Q127, scalar2=LN_EPS * Q127,
                                                op0=ALU.mult, op1=ALU.max)
                    nc.vector.reciprocal(so_c, gamo)
                    nc.vector.tensor_scalar_mul(deqo, gamo, gwo)
                    for sub in range(4):
                        oq8 = wop.tile([128, HL], I8, tag="oq8")
                        nc.gpsimd.tensor_scalar_mul(oq8, out_tok[:, sub, :], so_c[:, sub:sub + 1])
                        oqb_t = wop.tile([128, HL], BF16, tag="oqb")
                        nc.gpsimd.tensor_copy(oqb_t, oq8)
                        oqT = wop.tile([128, 2, 128], BF16, tag="oqT")
                        for oc in range(2):
                            trp = ps_tr.tile([128, 512], BF16, tag="tr")
                            nc.tensor.transpose(trp[:, 0:128], oqb_t[:, oc * 128:(oc + 1) * 128], ident)
                            nc.vector.tensor_copy(oqT[:, oc, :], trp[:, 0:128])
                        for cb in range(2):
                            mm = ps_mm.tile([128, 512], F32, tag="mm")
                            for oc in range(2):
                                nc.tensor.matmul(mm, oqT[:, oc, :], wo_t[:, oc, cb * 512:(cb + 1) * 512],
                                                 start=(oc == 0), stop=(oc == 1))
                            a_sb = wop.tile([128, 512], BF16, tag="a_sb")
                            nc.vector.tensor_scalar_mul(a_sb, mm, deqo[:, sub:sub + 1])
                            nc.sync.dma_start(
                                out=rs_in.ap()[qb][sub * 128:(sub + 1) * 128, cb * 512:(cb + 1) * 512],
                                in_=a_sb)
                    nc.gpsimd.collective_compute(
                        "ReduceScatter", ALU.add, replica_groups=RG,
                        ins=[rs_in.ap()[qb].opt()], outs=[rs_out.ap()[qb].opt()])
                    tick(float('inf'), allow_heavy=True, strict=False)

                def wchunk(qb, hb, ring):
                    wgc = wgs.tile([128, NCC, 512], FP8, tag="wg")
                    wv2c = wgs.tile([128, NCC, 512], FP8, tag="wv2")
                    nc.sync.dma_start(
                        out=wgc, in_=bass.AP(tensor=wg8.ap().tensor, offset=hb * 512,
                                             ap=[[HID, 128], [128 * HID, NCC], [1, 512]]))
                    nc.sync.dma_start(
                        out=wv2c, in_=bass.AP(tensor=wv28.ap().tensor, offset=hb * 512,
                                              ap=[[HID, 128], [128 * HID, NCC], [1, 512]]))
                    ring[(qb, hb)] = (wgc, wv2c)

                def dup2(ap_2d, n):
                    """[128, n] AP -> [128, 2, n] with a stride-0 middle dim."""
                    return bass.AP(tensor=ap_2d.tensor, offset=ap_2d.offset,
                                   ap=[ap_2d.ap[0], [0, 2], ap_2d.ap[-1]])

                strip = {}
                ring = {}

                def f_chain(qb):
                    """residual + LN2 + hi/lo planes -- DVE/Act/Pool only."""
                    wchunk(qb, 0, ring)
                    wchunk(qb, 1, ring)
                    wchunk(qb, 2, ring)
                    a_red = fp.tile([128, C], BF16, tag="ared")
                    nc.sync.dma_start(out=a_red, in_=rs_out.ap()[qb])
                    x2 = x_own[:, qb, :]
                    nc.vector.tensor_tensor(out=x2, in0=x2, in1=a_red, op=ALU.add)
                    if flags["b_o"]:
                        nc.gpsimd.tensor_tensor(out=x2, in0=x2, in1=bo_bc, op=ALU.add)
                    gam2 = fu.tile([128, 1], F32, tag="gam2")
                    h2_8 = ln_quant(fp, x2, aff2, gam2, as_bf=False)
                    hi8 = fp.tile([128, C], I8, tag="hi8")
                    nc.scalar.activation(out=hi8, in_=h2_8, func=AF.Copy, scale=1.0 / 16.0)
                    hi16 = fp.tile([128, C], BF16, tag="hi16")
                    nc.vector.tensor_scalar_mul(hi16, hi8, 16.0)
                    lo8 = fp.tile([128, C], BF16, tag="lo8")
                    nc.vector.scalar_tensor_tensor(out=lo8, in0=hi8, scalar=-16.0, in1=h2_8,
                                                   op0=ALU.mult, op1=ALU.add)
                    deq_g = fu.tile([128, 1], F32, tag="deqg")
                    deq_v = fu.tile([128, 1], F32, tag="deqv")
                    nc.vector.tensor_scalar_mul(deq_g, gam2, gwg)
                    nc.vector.tensor_scalar_mul(deq_v, gam2, gwv2)
                    st = {"x2": x2, "gam2": gam2, "hi16": hi16, "lo8": lo8,
                          "deq_g": deq_g, "deq_v": deq_v}
                    strip[qb] = st

                def f_planes(qb):
                    """transpose hi/lo planes to channel-major fp8."""
                    st = strip[qb]
                    h2pl = fp.tile([128, NCC, 2, 128], FP8, tag="h2pl")
                    for q4 in range(4):
                        cc0 = q4 * 2
                        trp = ps_tr.tile([128, 512], BF16, tag="tr")
                        for j in range(2):
                            cc = cc0 + j
                            nc.tensor.transpose(trp[:, (2 * j) * 128:(2 * j + 1) * 128],
                                                st["hi16"][:, cc * 128:(cc + 1) * 128], ident)
                            nc.tensor.transpose(trp[:, (2 * j + 1) * 128:(2 * j + 2) * 128],
                                                st["lo8"][:, cc * 128:(cc + 1) * 128], ident)
                        nc.vector.tensor_copy(
                            h2pl[:, cc0:cc0 + 2, :, :],
                            trp.rearrange("p (a b f) -> p a b f", a=2, b=2))
                    st["h2pl"] = h2pl
                    st["u"] = fu.tile([128, HID], BF16, tag="u", name=f"u{qb}")
                    st["ucols"] = fu.tile([128, 8], F32, tag="ucols", name=f"ucols{qb}")

                def f_hb(qb, hb):
                    """one 512-hidden block of gate/val (DoubleRow) + u."""
                    st = strip[qb]
                    wgc, wv2c = ring.pop((qb, hb))
                    h2pl, deq_g, deq_v = st["h2pl"], st["deq_g"], st["deq_v"]
                    gmm = ps_mm.tile([128, 512], F32, tag="mm")
                    for cc in range(NCC):
                        nc.tensor.matmul(gmm, h2pl[:, cc, :, :], dup2(wgc[:, cc, :], 512),
                                         start=(cc == 0), stop=(cc == NCC - 1),
                                         perf_mode=mybir.MatmulPerfMode.DoubleRow)
                    sil = fp.tile([128, 512], BF16, tag="sil")
                    if flags["b_ffn"]:
                        gd_f = fp.tile([128, 512], F32, tag="gdf")
                        nc.vector.scalar_tensor_tensor(
                            out=gd_f, in0=gmm, scalar=deq_g[:, 0:1],
                            in1=bg_bc[:, hb * 512:(hb + 1) * 512], op0=ALU.mult, op1=ALU.add)
                        nc.scalar.activation(out=sil, in_=gd_f, func=AF.Silu)
                    else:
                        nc.scalar.activation(out=sil, in_=gmm, func=AF.Silu, scale=deq_g[:, 0:1])
                    vmm = ps_mm.tile([128, 512], F32, tag="mm")
                    for cc in range(NCC):
                        nc.tensor.matmul(vmm, h2pl[:, cc, :, :], dup2(wv2c[:, cc, :], 512),
                                         start=(cc == 0), stop=(cc == NCC - 1),
                                         perf_mode=mybir.MatmulPerfMode.DoubleRow)
                    u_sl = st["u"][:, hb * 512:(hb + 1) * 512]
                    if flags["b_ffn"]:
                        vd_f = fp.tile([128, 512], F32, tag="vdf")
                        nc.vector.scalar_tensor_tensor(
                            out=vd_f, in0=vmm, scalar=deq_v[:, 0:1],
                            in1=bv2_bc[:, hb * 512:(hb + 1) * 512], op0=ALU.mult, op1=ALU.add)
                        nc.vector.tensor_tensor(out=u_sl, in0=vd_f, in1=sil, op=ALU.mult)
                    else:
                        nc.vector.scalar_tensor_tensor(
                            out=u_sl, in0=vmm, scalar=deq_v[:, 0:1],
                            in1=sil, op0=ALU.mult, op1=ALU.mult)
                    nc.vector.tensor_reduce(out=st["ucols"][:, hb:hb + 1], in_=u_sl,
                                            axis=mybir.AxisListType.X, op=ALU.max,
                                            apply_absolute_value=True)
                    if hb + 3 <= 7:
                        wchunk(qb, hb + 3, ring)

                def f_p2q(qb):
                    """u quantization scalars + int8 cast -- no PE."""
                    st = strip[qb]
                    ugraw = fp.tile([128, 1], F32, tag="ugraw")
                    nc.vector.tensor_reduce(out=ugraw, in_=st["ucols"], axis=mybir.AxisListType.X,
                                            op=ALU.max)
                    gamu = fu.tile([128, 1], F32, tag="gamu")
                    nc.vector.tensor_scalar(out=gamu, in0=ugraw, scalar1=Q127, scalar2=LN_EPS * Q127,
                                            op0=ALU.mult, op1=ALU.max)
                    surec = fu.tile([128, 1], F32, tag="surec")
                    nc.vector.reciprocal(surec, gamu)
                    dequ = fu.tile([128, 1], F32, tag="dequ")
                    nc.vector.tensor_scalar_mul(dequ, gamu, gwu)
                    st["dequ"] = dequ
                    u8 = fup.tile([128, HID], I8, tag="u8", name=f"u8_{qb}")
                    nc.scalar.activation(out=u8, in_=st["u"], func=AF.Copy, scale=surec[:, 0:1])
                    u_bf = fup.tile([128, HID], BF16, tag="ubf", name=f"ubf{qb}")
                    nc.gpsimd.tensor_copy(u_bf[:, 0:HID // 2], u8[:, 0:HID // 2])
                    nc.vector.tensor_copy(u_bf[:, HID // 2:], u8[:, HID // 2:])
                    st["ubf"] = u_bf
                    st["uT"] = fup.tile([128, NHC, 128], BF16, tag="uT", name=f"uT{qb}")

                def f_p2tr(qb, pair):
                    """two transpose groups of the quantized u."""
                    st = strip[qb]
                    for grp8 in (2 * pair, 2 * pair + 1):
                        trp = ps_tr.tile([128, 512], BF16, tag="tr")
                        for j in range(4):
                            hc = grp8 * 4 + j
                            nc.tensor.transpose(trp[:, j * 128:(j + 1) * 128],
                                                st["ubf"][:, hc * 128:(hc + 1) * 128], ident)
                        nc.vector.tensor_copy(st["uT"][:, grp8 * 4:(grp8 + 1) * 4, :], trp)

                def f_down(qb, cb):
                    """one 512-col block of the down-proj + residual out."""
                    st = strip[qb]
                    fmm = ps_mm.tile([128, 512], F32, tag="mm")
                    for hc in range(NHC):
                        nc.tensor.matmul(fmm, st["uT"][:, hc, :], wu_t[:, hc, cb * 512:(cb + 1) * 512],
                                         start=(hc == 0), stop=(hc == NHC - 1))
                    yt = fp.tile([128, 512], F32, tag="yt")
                    nc.vector.scalar_tensor_tensor(
                        out=yt, in0=fmm, scalar=st["dequ"][:, 0:1],
                        in1=st["x2"][:, cb * 512:(cb + 1) * 512], op0=ALU.mult, op1=ALU.add)
                    if flags["b_out"]:
                        nc.gpsimd.tensor_tensor(out=yt, in0=yt,
                                                in1=bout_bc[:, cb * 512:(cb + 1) * 512], op=ALU.add)
                    nc.sync.dma_start(out=y.ap()[qb][:, cb * 512:(cb + 1) * 512], in_=yt)

                def queue_p1(qb, r):
                    pending.append((r, 0, True, lambda: f_chain(qb)))
                    pending.append((r, 1400, False, lambda: f_planes(qb)))
                    for hb in range(8):
                        pending.append((r, 1800, True, lambda hb=hb: f_hb(qb, hb)))

                def queue_p2(qb, r):
                    pending.append((r, 0, True, lambda: f_p2q(qb)))
                    for pair in range(4):
                        pending.append((r + 1, 1200, False, lambda pair=pair: f_p2tr(qb, pair)))
                    for cb in range(2):
                        pending.append((r + 1, 6900, False, lambda cb=cb: f_down(qb, cb)))

                # fine-grained schedule: attention runs 3,2,1,0 (Act-bound
                # lockstep); FFN thunks fill the spare PE cycles as soon as
                # each block's ReduceScatter lands
                # slot map: 5 ticks per attn block (4 hd + 1 post):
                # attn(3): 1-5, attn(2): 6-10, attn(1): 11-15, attn(0): 16-20.
                # A strip's chain may start ~3 slots after its RS fires.
                queue_p1(3, 15)
                queue_p1(2, 20)
                queue_p2(3, 20)
                queue_p1(1, 996)
                queue_p2(2, 996)
                queue_p1(0, 997)
                queue_p2(1, 997)
                queue_p2(0, 998)
                attn_block(3)
                attn_block(2)
                attn_block(1)
                attn_block(0)
                SLOT[0] = 10 ** 6
                run_fillers(float("inf"))

    nc.finalize()
    return nc


def _get_program(gw, flags):
    global LAST_PROGRAM
    key = (tuple(np.float32(v).item() for v in gw), tuple(sorted(flags.items())))
    with _PROGRAM_LOCK:
        if key not in _PROGRAMS:
            _PROGRAMS[key] = build_program(gw, flags)
    LAST_PROGRAM = _PROGRAMS[key]
    return LAST_PROGRAM


def kernel(**inputs):
    global LAST_RESULTS
    f32 = lambda a: np.ascontiguousarray(np.asarray(a), dtype=np.float32)
    x = f32(inputs["x"])
    wq, wk, wv, wo = f32(inputs["wq"]), f32(inputs["wk"]), f32(inputs["wv"]), f32(inputs["wo"])
    wgate, wval, wout = f32(inputs["wgate"]), f32(inputs["wval"]), f32(inputs["wout"])

    ws = (wq, wk, wv, wo, wgate, wval, wout)
    gw = [max(np.mean(np.abs(w), dtype=np.float32), np.float32(1e-5)) for w in ws]

    def tern(w, g):
        return np.clip(np.round(w / g), -1, 1).astype(ml_dtypes.float8_e4m3fn)

    tq, tk, tv, to, tg, tv2, tu = [tern(w, g) for w, g in zip(ws, gw)]

    flags = {
        "ln1_aff": not (np.all(inputs["ln1_g"] == 1) and np.all(inputs["ln1_b"] == 0)),
        "ln2_aff": not (np.all(inputs["ln2_g"] == 1) and np.all(inputs["ln2_b"] == 0)),
        "b_o": not np.all(inputs["bo"] == 0),
        "b_ffn": not (np.all(inputs["bgate"] == 0) and np.all(inputs["bval"] == 0)),
        "b_out": not np.all(inputs["bout"] == 0),
    }
    assert np.all(inputs["bq"] == 0) and np.all(inputs["bk"] == 0) and np.all(inputs["bv"] == 0), \
        "nonzero qkv biases not supported"

    in_maps = []
    for c in range(N_CORES):
        b, g = c // G, c % G
        strips = np.stack([x[b, qb * 512 + 128 * g: qb * 512 + 128 * (g + 1), :]
                           for qb in range(NTB)])
        m = {
            "x_sl": f32(x[b]),
            "x_strips": f32(strips),
            "wq8": np.ascontiguousarray(tq.T[:, g * HL:(g + 1) * HL]),
            "wk8": np.ascontiguousarray(tk.T[:, g * HL:(g + 1) * HL]),
            "wv8": np.ascontiguousarray(tv.T[:, g * HL:(g + 1) * HL]),
            "wo8": np.ascontiguousarray(to.T[g * HL:(g + 1) * HL, :]),
            "wg8": np.ascontiguousarray(tg.T),
            "wv28": np.ascontiguousarray(tv2.T),
            "wu8": np.ascontiguousarray(tu.T),
            "ln1g": f32(inputs["ln1_g"]),
            "ln1b": f32(inputs["ln1_b"]),
            "ln2g": f32(inputs["ln2_g"]),
            "ln2b": f32(inputs["ln2_b"]),
            "bo_f": f32(inputs["bo"]),
            "bg_s": f32(inputs["bgate"]),
            "bv2_s": f32(inputs["bval"]),
            "bout_f": f32(inputs["bout"]),
        }
        in_maps.append(m)

    nc = _get_program(gw, flags)
    trace = bool(int(os.environ.get("KERNEL_TRACE", "0")))
    res = run_bass_kernel_spmd(nc, in_maps, core_ids=list(range(N_CORES)), trace=trace)
    LAST_RESULTS = res

    out = np.empty((B, T, C), dtype=np.float32)
    for c in range(N_CORES):
        b, g = c // G, c % G
        yv = res.results[c]["y"]  # [NTB, 128, C]
        for qb in range(NTB):
            out[b, qb * 512 + 128 * g: qb * 512 + 128 * (g + 1), :] = yv[qb]
    return out
